# revision 38
# baseline (speedup 1.0000x reference)
"""MultiHeadAttention Trainium2 kernel (8 NeuronCores, SPMD).

Sharding: data-parallel over batch (B=2), tensor-parallel over heads
(16 heads -> 4 per core).  Core c handles batch b=c//4, head group
g=c%4 (heads 4g..4g+3).  Wq/Wk/Wv are split column-wise, Wo row-wise;
the per-core Wo partial outputs are summed on the host (replaces the
all-reduce).

All DRAM inputs are host-preswizzled into [128, N] layouts in the
order the kernel consumes them, so every load is a contiguous
column-range DMA with large descriptors (the wire is the front's
critical path).

Device dataflow per core (bf16 matmuls, f32 PSUM accumulation):
  Front: kT = Wk_g^T x^T (k-outer, paced by the two xk half-DMAs),
  v keys 0-11 in two 6-bank PSUM waves, q chunk 0.  v keys 12-15 and
  q chunks 1-3 are deferred into the attention pipeline as <=2-matmul
  items (their DMAs land later).
    qTz = zero-padded q [128, 4096] per head-pair pr; block j=2c+hh
    holds head hh's q rows for chunk c, other 64 rows zero -> the
    logits matmul is a full K=128 contraction (both heads in one
    N=512 pair), no PE tiling-mode switches.
  Attention: one flat pipeline over (c, pr, st):
    lg[:, hh*512:+512] = kT-block^T qTz-block
    p = exp(lg) bf16: ScalarE activation (scale=1/8 fused); on st in
        FAST_STS a DVE int16-Schraudolph fast exp (bitcast bf16)
        offloads the ScalarE exp bottleneck (~0.9% extra rel err).
    accT[65, s_q] += vaug_h^T p     (row 64 = softmax denominator)
    octT = acc[0:64]/acc[64] via recip+broadcast, then Wo partials
    (2-matmul items) stream out as bf16 (summed on host in f32).
"""

import sys

import numpy as np

sys.path.insert(0, "/opt/trn_rl_repo")

import ml_dtypes  # noqa: E402

import concourse.bass as bass  # noqa: E402
import concourse.mybir as mybir  # noqa: E402
import concourse.tile as tile  # noqa: E402
from concourse import bacc  # noqa: E402
from concourse.bass import ts  # noqa: E402
from concourse.bass_utils import run_bass_kernel_spmd  # noqa: E402

S = 2048  # sequence length (S * X)
D = 1024  # model dim
H = 16  # total heads
HL = 4  # heads per core
DH = 64  # head dim
DQ = HL * DH  # per-core projection width = 256
NK = D // 128  # K tiles for projections = 8
NST = S // 128  # s_k tiles = 16
NCH = S // 512  # s_q chunks = 4

BF16 = mybir.dt.bfloat16
F32 = mybir.dt.float32
I16 = mybir.dt.int16

# v-projection waves (keys 0-5 / 6-11 inline, 12-15 injected into the
# attention pipeline); xv DRAM is laid out wave-major.
V_WAVES = ((0, 6), (6, 12), (12, 16))

# DVE fast-exp (int16 Schraudolph): p = 2^(lg * 0.125 * log2 e) via
# bf16-bit arithmetic: bits = round(lg * S1 + S2), reinterpreted bf16.
# numpy-validated on the real inputs (3/16 tiles -> rel_err ~1.0e-2,
# gate 2e-2); sts chosen away from (c,pr) closes where the DVE queue
# is busy with normalization.
# DVE int16-Schraudolph fast-exp (numerically validated at 1.07e-2 with
# sts (4,8,12)) measured net-negative on time in every configuration:
# the DVE FIFO latency stalls the AV stream / the (c,pr) close chain.
FAST_STS = ()
EXP_S1 = 16.0 / np.log(2.0)
EXP_S2 = 16256.0 - 128.0 * 0.0433

TRACE = False
LAST_RESULTS = None

_BUILT = None


def _xv_slice(xv_all, k, st):
    for w0, w1 in V_WAVES:
        if w0 <= st < w1:
            off = w0 * 128 * NK + k * (w1 - w0) * 128 + (st - w0) * 128
            return xv_all[:, off : off + 128]
    raise AssertionError


def _xq_slice(xq_all, k, c):
    off = c * 512 * NK + k * 512
    return xq_all[:, off : off + 512]


def _emit(ctx, tc, io):
    nc = tc.nc
    xq, xk, xv = io["xqT"], io["xkT"], io["xvT"]
    wq, wk, wv, wo = io["wq"], io["wk"], io["wv"], io["wo"]
    bq, bk, bv = io["bq"], io["bk"], io["bv"]
    out = io["out"]

    consts = ctx.enter_context(tc.tile_pool(name="consts", bufs=1))
    xin = ctx.enter_context(tc.tile_pool(name="xin", bufs=1))
    qk = ctx.enter_context(tc.tile_pool(name="qk", bufs=1))
    ptiles = ctx.enter_context(tc.tile_pool(name="ptiles", bufs=7))
    norm = ctx.enter_context(tc.tile_pool(name="norm", bufs=3))
    osb_pool = ctx.enter_context(tc.tile_pool(name="osb", bufs=3))

    wq_all = consts.tile([128, NK * DQ], BF16, tag="wq", name="wq_all")
    wk_all = consts.tile([128, NK * DQ], BF16, tag="wk", name="wk_all")
    wv_all = consts.tile([128, NK * DQ], BF16, tag="wv", name="wv_all")
    wo_all = consts.tile([128, 2 * D], BF16, tag="wo", name="wo_all")
    xq_all = xin.tile([128, NK * S], BF16, tag="xq", name="xq_all")
    xk_all = xin.tile([128, NK * S], BF16, tag="xk", name="xk_all")
    xv_all = xin.tile([128, NK * S], BF16, tag="xv", name="xv_all")
    wq_t = [wq_all[:, k * DQ : (k + 1) * DQ] for k in range(NK)]
    wk_t = [wk_all[:, k * DQ : (k + 1) * DQ] for k in range(NK)]
    wv_t = [wv_all[:, k * DQ : (k + 1) * DQ] for k in range(NK)]
    wo_t = [wo_all[:, k * D : (k + 1) * D] for k in range(2)]
    xk_t = [xk_all[:, k * S : (k + 1) * S] for k in range(NK)]
    # bq/bk as [128, 2] per-partition scalars (col j = dq 128j..128j+127)
    bq_sb = consts.tile([128, 2], F32, tag="bq", name="bq_sb")
    bk_sb = consts.tile([128, 2], F32, tag="bk", name="bk_sb")
    bv_sb = consts.tile([128, DQ], F32, tag="bv", name="bv_sb")

    kT = [qk.tile([128, S], BF16, tag=f"kT{m}", name=f"kT{m}") for m in range(2)]
    qTz = [qk.tile([128, 2 * S], BF16, tag=f"qTz{m}", name=f"qTz{m}") for m in range(2)]
    octT = [qk.tile([128, S], BF16, tag=f"octT{m}", name=f"octT{m}") for m in range(2)]
    vaug = [qk.tile([128, HL * (DH + 1)], BF16, tag=f"vaug{st}", name=f"vaug{st}") for st in range(NST)]

    # ---- t=0 housekeeping on otherwise-idle engines ----
    # exp ACT-table preload: first real exp otherwise pays ~2.7us mid-kernel
    dm_in = consts.tile([128, 8], F32, tag="dm_in", name="dm_in")
    dm_out = consts.tile([128, 8], BF16, tag="dm_out", name="dm_out")
    nc.vector.memset(dm_in[:], 0.0)
    nc.scalar.activation(dm_out[:], dm_in[:], mybir.ActivationFunctionType.Exp, scale=0.125)
    # PE warmup: dummy matmuls release the HAM clock gate (4/8 -> 8/8)
    # and bridge until the first xk half lands.
    wu_sb = consts.tile([128, 512], BF16, tag="wu", name="wu_sb")
    nc.vector.memset(wu_sb[:], 1.0)
    # zero-pad qTz and set the vaug ones-columns up front
    for m in range(2):
        nc.vector.memset(qTz[m][:], 0.0)
    for st in range(NST):
        for h in range(HL):
            nc.vector.memset(vaug[st][:, h * 65 + 64 : h * 65 + 65], 1.0)

    # Tiny bias DMAs on the gpsimd (SWDGE) queue; all bulk traffic on the
    # sync (HWDGE) queue in consumption order.
    nc.gpsimd.dma_start(
        out=bk_sb[:], in_=bass.AP(tensor=bk.tensor, offset=bk.offset, ap=[[1, 128], [128, 2]])
    )
    nc.gpsimd.dma_start(
        out=bq_sb[:], in_=bass.AP(tensor=bq.tensor, offset=bq.offset, ap=[[1, 128], [128, 2]])
    )
    nc.gpsimd.dma_start(
        out=bv_sb[:], in_=bass.AP(tensor=bv.tensor, offset=bv.offset, ap=[[0, 128], [1, DQ]])
    )

    nc.sync.dma_start(wk_all[:], wk[:, :])
    for half in range(2):
        sl = slice(half * 4 * S, (half + 1) * 4 * S)
        nc.sync.dma_start(xk_all[:, sl], xk[:, sl])
    nc.sync.dma_start(wv_all[:], wv[:, :])
    nc.sync.dma_start(xv_all[:, 0:6144], xv[:, 0:6144])  # keys 0-5
    nc.sync.dma_start(wq_all[:], wq[:, :])
    nc.sync.dma_start(xq_all[:, 0:4096], xq[:, 0:4096])  # q chunk 0
    nc.sync.dma_start(xv_all[:, 6144:12288], xv[:, 6144:12288])  # keys 6-11
    nc.sync.dma_start(xv_all[:, 12288:16384], xv[:, 12288:16384])  # keys 12-15
    for c in range(1, NCH):
        sl = slice(c * 4096, (c + 1) * 4096)
        nc.sync.dma_start(xq_all[:, sl], xq[:, sl])
    nc.sync.dma_start(wo_all[:], wo[:, :])

    # ---- front: one 8-bank PSUM pool; k-projection k-outermost (each
    # k-pass runs as soon as its xk half lands), then v waves (ONE
    # accumulation group per bank -- start_tensor_calc claims a whole
    # 2KB zero-region), then q chunk 0.
    groups = [(m, c) for m in range(2) for c in range(NCH)]

    def q_bias(m, c, ps, eng):
        for hh in range(2):
            dst = qTz[m][ts(hh, 64), (2 * c + hh) * 512 : (2 * c + hh) * 512 + 512]
            src = ps[ts(hh, 64), :]
            b = bq_sb[ts(hh, 64), m : m + 1]
            if eng is nc.scalar:
                eng.add(dst, src, b)
            else:
                eng.tensor_scalar_add(dst, src, b)

    with tc.tile_pool(name="psf", bufs=8, space="PSUM") as psf:
        # 36 matmuls ~= 8 cold (3.4us, releases the HAM 4/8 gate) + 28
        # warm, ending ~20us -- bridging all the way to the first
        # xk-half's wire completion so the k-projection never starts
        # re-throttled (a >3.4us PE idle re-arms the clock gate).
        wu_ps = psf.tile([128, 512], F32, tag="ps", name="wu_ps")
        for _ in range(36):
            nc.tensor.matmul(wu_ps[:], wu_sb[:, 0:128], wu_sb[:], start=True, stop=True)
        kps = {g: psf.tile([128, 512], F32, tag="ps", name=f"kps{g}") for g in groups}
        for k in range(NK):
            for m, c in groups:
                nc.tensor.matmul(
                    kps[(m, c)][:],
                    wk_t[k][:, ts(m, 128)],
                    xk_t[k][:, ts(c, 512)],
                    start=(k == 0),
                    stop=(k == NK - 1),
                )
        for i, (m, c) in enumerate(groups):
            eng = nc.scalar if i % 2 else nc.vector
            if eng is nc.scalar:
                eng.add(kT[m][:, ts(c, 512)], kps[(m, c)][:], bk_sb[:, m : m + 1])
            else:
                eng.tensor_scalar_add(kT[m][:, ts(c, 512)], kps[(m, c)][:], bk_sb[:, m : m + 1])

        def v_wave(wave):
            vps = {st: psf.tile([128, 512], F32, tag="ps", name=f"vps{st}") for st in wave}
            for k in range(NK):
                for st in wave:
                    nc.tensor.matmul(
                        vps[st][:, 0:256],
                        _xv_slice(xv_all, k, st),
                        wv_t[k][:],
                        start=(k == 0),
                        stop=(k == NK - 1),
                    )
            for st in wave:
                for h in range(HL):
                    nc.vector.tensor_add(
                        vaug[st][:, h * 65 : h * 65 + 64],
                        vps[st][:, ts(h, DH)],
                        bv_sb[:, ts(h, DH)],
                    )

        v_wave(range(0, 6))
        for m in range(2):
            qc0 = psf.tile([128, 512], F32, tag="ps", name=f"qc0_{m}")
            for k in range(NK):
                nc.tensor.matmul(
                    qc0[:],
                    wq_t[k][:, ts(m, 128)],
                    _xq_slice(xq_all, k, 0),
                    start=(k == 0),
                    stop=(k == NK - 1),
                )
            q_bias(m, 0, qc0[:], nc.scalar if m else nc.vector)
        v_wave(range(6, 12))

    # ---- attention ----
    psum_lg = ctx.enter_context(tc.tile_pool(name="psum_lg", bufs=2, space="PSUM"))
    psum_mm = ctx.enter_context(tc.tile_pool(name="psum_mm", bufs=4, space="PSUM"))

    steps = [(c, pr, st) for c in range(NCH) for pr in range(2) for st in range(NST)]
    LAG = 2

    # deferred PE work queue: (min_step, fn) items, each <= 2 matmuls
    # (~430ns); up to 2 pops/step early on (v tail + q chunks), 1 later,
    # so the exp stream stays fed (base step = 4 matmuls)
    pe_queue = []
    acc_map = {}
    p_map = {}

    def defer_q_group(m, c, min_step):
        state = {}

        def pair(kk):
            def run():
                if kk == 0:
                    state["ps"] = psum_mm.tile([128, 512], F32, tag="mm", name="mm")
                for k in (kk, kk + 1):
                    nc.tensor.matmul(
                        state["ps"][:],
                        wq_t[k][:, ts(m, 128)],
                        _xq_slice(xq_all, k, c),
                        start=(k == 0),
                        stop=(k == NK - 1),
                    )
                if kk == NK - 2:
                    q_bias(m, c, state["ps"][:], nc.vector)

            return run

        pe_queue.extend((min_step, pair(kk)) for kk in range(0, NK, 2))

    def defer_v_st(st):
        state = {}

        def pair(kk):
            def run():
                if kk == 0:
                    state["ps"] = psum_mm.tile([128, 512], F32, tag="mm", name="mm")
                for k in (kk, kk + 1):
                    nc.tensor.matmul(
                        state["ps"][:, 0:256],
                        _xv_slice(xv_all, k, st),
                        wv_t[k][:],
                        start=(k == 0),
                        stop=(k == NK - 1),
                    )
                if kk == NK - 2:
                    for h in range(HL):
                        nc.vector.tensor_add(
                            vaug[st][:, h * 65 : h * 65 + 64],
                            state["ps"][:, ts(h, DH)],
                            bv_sb[:, ts(h, DH)],
                        )

            return run

        pe_queue.extend((0, pair(kk)) for kk in range(0, NK, 2))

    def wo_item(c, smt, ncho, k, state, on_act=False):
        def run():
            row = c * 512 + smt * 128
            if k == 0:
                state["ps"] = psum_mm.tile([128, 512], F32, tag="mm", name="mm")
            nc.tensor.matmul(
                state["ps"][:],
                octT[k][:, row : row + 128],
                wo_t[k][:, ts(ncho, 512)],
                start=(k == 0),
                stop=(k == 1),
            )
            if k == 1:
                osb = osb_pool.tile([128, 512], BF16, tag="osb", name="osb")
                if on_act and smt % 2:  # tail: split copies Scalar/Vector
                    nc.scalar.copy(osb[:], state["ps"][:])
                else:
                    nc.vector.tensor_copy(osb[:], state["ps"][:])
                dma = nc.gpsimd if (smt + ncho) % 2 else nc.sync
                dma.dma_start(out[row : row + 128, ts(ncho, 512)], osb[:])

        return run

    def defer_wo_group(c, smt, ncho, on_act=False):
        state = {}
        pe_queue.append((0, wo_item(c, smt, ncho, 0, state, on_act)))
        pe_queue.append((0, wo_item(c, smt, ncho, 1, state, on_act)))

    def emit_front(c, pr, st):
        lg = psum_lg.tile([128, 1024], F32, tag="lg", name="lg")
        kblk = kT[pr][:, ts(st, 128)]
        for hh in range(2):
            nc.tensor.matmul(
                lg[:, ts(hh, 512)],
                kblk,
                qTz[pr][:, (2 * c + hh) * 512 : (2 * c + hh) * 512 + 512],
                start=True,
                stop=True,
            )
        p = ptiles.tile([128, 1024], BF16, tag="p", name="p")
        if st in FAST_STS:
            nc.vector.tensor_scalar(
                p[:].bitcast(I16),
                lg[:],
                EXP_S1,
                EXP_S2,
                mybir.AluOpType.mult,
                mybir.AluOpType.add,
            )
        else:
            nc.scalar.activation(p[:], lg[:], mybir.ActivationFunctionType.Exp, scale=0.125)
        p_map[(c, pr, st)] = p

    def emit_acc(c, pr, st, pos):
        if pos == 0:
            acc_map[(c, pr)] = [
                psum_mm.tile([65, 512], F32, tag="mm", name="mm") for _ in range(2)
            ]
        acc = acc_map[(c, pr)]
        pp = p_map.pop((c, pr, st))
        for hh in range(2):
            h = 2 * pr + hh
            nc.tensor.matmul(
                acc[hh][:],
                vaug[st][:, h * 65 : h * 65 + 65],
                pp[:, ts(hh, 512)],
                start=(pos == 0),
                stop=(pos == NST - 1),
            )
        if pos == NST - 1:
            # normalize: octT[pr][64*hh, chunk c] = acc[0:64] / acc[64].
            # Fast copies release the PSUM accumulators (un on ScalarE,
            # den on DVE, racing -- the next pair's AV needs the slots);
            # broadcast + approx-reciprocal + mul run off the critical
            # path.
            for hh in range(2):
                un = norm.tile([64, 512], BF16, tag="un", name="un")
                nc.vector.tensor_copy(un[:], acc[hh][0:64, :])
                den = norm.tile([1, 512], F32, tag="den", name="den")
                nc.vector.tensor_copy(den[:], acc[hh][64:65, :])
                bc = norm.tile([64, 512], F32, tag="bcast", name="bcast")
                nc.gpsimd.partition_broadcast(bc[:], den[:])
                rbc = norm.tile([64, 512], F32, tag="rbc", name="rbc")
                nc.vector.reciprocal_approx_fast(rbc[:], bc[:])
                nc.vector.tensor_mul(octT[pr][ts(hh, 64), ts(c, 512)], un[:], rbc[:])
            del acc_map[(c, pr)]
            if pr == 1:
                on_act = c == NCH - 1
                for smt in range(4):
                    for ncho in range(2):
                        defer_wo_group(c, smt, ncho, on_act)

    # deferred work, in deadline order: v keys 12-15 (needed from step
    # ~14), then q chunks 1-3 (needed from step 32c; min_step gates on
    # the staggered xq DMAs)
    for st in range(12, 16):
        defer_v_st(st)
    for c in range(1, NCH):
        for m in range(2):
            defer_q_group(m, c, min_step=4 * c)

    # accumulation-stream order: fast-exp tiles pushed ~3 positions later
    # (capped so the close position stays a ScalarE tile)
    acc_order = sorted(
        range(NST), key=lambda st: min(st + 3.5, NST - 1.5) if st in FAST_STS else st
    )
    acc_steps = [
        (c, pr, st, pos)
        for c in range(NCH)
        for pr in range(2)
        for pos, st in enumerate(acc_order)
    ]

    for i, s in enumerate(steps):
        c, pr, st = s
        emit_front(c, pr, st)
        if i >= LAG:
            emit_acc(*acc_steps[i - LAG])
        budget = 2 if i < 26 else 1
        while budget and pe_queue and pe_queue[0][0] <= i:
            pe_queue.pop(0)[1]()
            budget -= 1

    for i in range(len(steps) - LAG, len(steps)):
        emit_acc(*acc_steps[i])

    for _, g in pe_queue:
        g()


def _build():
    global _BUILT
    if _BUILT is not None:
        return _BUILT
    nc = bacc.Bacc(
        "TRN2",
        target_bir_lowering=False,
        debug=False,
        enable_asserts=False,
        num_devices=8,
    )
    io = {}
    io["xqT"] = nc.dram_tensor("xqT", [128, NK * S], BF16, kind="ExternalInput").ap()
    io["xkT"] = nc.dram_tensor("xkT", [128, NK * S], BF16, kind="ExternalInput").ap()
    io["xvT"] = nc.dram_tensor("xvT", [128, NK * S], BF16, kind="ExternalInput").ap()
    io["wq"] = nc.dram_tensor("wq", [128, NK * DQ], BF16, kind="ExternalInput").ap()
    io["wk"] = nc.dram_tensor("wk", [128, NK * DQ], BF16, kind="ExternalInput").ap()
    io["wv"] = nc.dram_tensor("wv", [128, NK * DQ], BF16, kind="ExternalInput").ap()
    io["wo"] = nc.dram_tensor("wo", [128, 2 * D], BF16, kind="ExternalInput").ap()
    io["bq"] = nc.dram_tensor("bq", [DQ], F32, kind="ExternalInput").ap()
    io["bk"] = nc.dram_tensor("bk", [DQ], F32, kind="ExternalInput").ap()
    io["bv"] = nc.dram_tensor("bv", [DQ], F32, kind="ExternalInput").ap()
    io["out"] = nc.dram_tensor("out", [S, D], BF16, kind="ExternalOutput").ap()
    from contextlib import ExitStack

    with tile.TileContext(nc) as tc, ExitStack() as ctx:
        _emit(ctx, tc, io)
    nc.compile()
    _BUILT = nc
    return nc


def _swizzle_k(a, width):
    # [NK*128, width] -> [128, NK*width] with tile k at cols [k*width, ...)
    nk = a.shape[0] // 128
    return np.ascontiguousarray(
        a.reshape(nk, 128, width).transpose(1, 0, 2).reshape(128, nk * width)
    )


def _swizzle_xv(a):
    # [1024, S] -> [128, NK*S] wave-major: wave w cols, within: k-major
    parts = []
    for w0, w1 in V_WAVES:
        blk = a[:, w0 * 128 : w1 * 128]  # [1024, width]
        parts.append(_swizzle_k(blk, (w1 - w0) * 128))
    return np.ascontiguousarray(np.concatenate(parts, axis=1))


def _swizzle_xq(a):
    # [1024, S] -> [128, NK*S] chunk-major: chunk c cols, within: k-major
    parts = [_swizzle_k(a[:, c * 512 : (c + 1) * 512], 512) for c in range(NCH)]
    return np.ascontiguousarray(np.concatenate(parts, axis=1))


def kernel(**inputs):
    global LAST_RESULTS
    bf16 = ml_dtypes.bfloat16
    query = np.asarray(inputs["query"], np.float32).reshape(2, S, D)
    key = np.asarray(inputs["key"], np.float32).reshape(2, S, D)
    value = np.asarray(inputs["value"], np.float32).reshape(2, S, D)
    Wq = np.asarray(inputs["Wq"], np.float32)
    Wk = np.asarray(inputs["Wk"], np.float32)
    Wv = np.asarray(inputs["Wv"], np.float32)
    Wo = np.asarray(inputs["Wo"], np.float32)
    bq = np.asarray(inputs["bq"], np.float32)
    bk = np.asarray(inputs["bk"], np.float32)
    bv = np.asarray(inputs["bv"], np.float32)
    bo = np.asarray(inputs["bo"], np.float32)

    xT = {}
    for b in range(2):
        xq_b = np.ascontiguousarray(query[b].T).astype(bf16)
        xk_b = np.ascontiguousarray(key[b].T).astype(bf16)
        xv_b = np.ascontiguousarray(value[b].T).astype(bf16)
        xT[("q", b)] = _swizzle_xq(xq_b)
        xT[("k", b)] = _swizzle_k(xk_b, S)
        xT[("v", b)] = _swizzle_xv(xv_b)

    in_maps = []
    for c in range(8):
        b, g = c // 4, c % 4
        sl = slice(g * DQ, (g + 1) * DQ)
        in_maps.append(
            {
                "xqT": xT[("q", b)],
                "xkT": xT[("k", b)],
                "xvT": xT[("v", b)],
                "wq": _swizzle_k(np.ascontiguousarray(Wq[:, sl]).astype(bf16), DQ),
                "wk": _swizzle_k(np.ascontiguousarray(Wk[:, sl]).astype(bf16), DQ),
                "wv": _swizzle_k(np.ascontiguousarray(Wv[:, sl]).astype(bf16), DQ),
                "wo": _swizzle_k(np.ascontiguousarray(Wo[sl, :]).astype(bf16), D),
                "bq": np.ascontiguousarray(bq[sl]),
                "bk": np.ascontiguousarray(bk[sl]),
                "bv": np.ascontiguousarray(bv[sl]),
            }
        )

    nc = _build()
    res = run_bass_kernel_spmd(
        nc, in_maps, core_ids=list(range(8)), trace=TRACE
    )
    LAST_RESULTS = res

    full = np.zeros((2, S, D), np.float32)
    for c in range(8):
        full[c // 4] += np.asarray(res.results[c]["out"], np.float32)
    full += bo[None, None, :]
    return full.reshape(2, S, 1, D)
